# revision 38
# baseline (speedup 1.0000x reference)
"""Inverse STFT (nn_InverseSTFT) as a Bass/Tile kernel on 8 TRN2 NeuronCores.

Math
----
Reference: full spectrum via conjugate symmetry (F = 1024), IDFT to ytmp,
overlap_add(hop=256), window-sum normalize, trim n_fft//2.

This kernel folds BOTH the conjugate symmetry AND the overlap-add into the
basis. With w = 256j + r the DFT phase satisfies
    e^{i 2pi f (256j+r)/1024} = i^{f j} * e^{i 2pi f r/1024},
so the 4-frame overlap-add sum becomes a per-frequency 4-tap filter along
frames with coefficients i^{f j} in {1, i, -1, -i} (fixed per class
c = f mod 4), followed by ONE K=1024 matmul per output segment:
    y[256 s + r] = sum_f cosb[f,r]*U[f,s] + sinb[f,r]*V[f,s]
i.e. 1024 MACs per output sample instead of 4096 (4x less PE work).

Frequencies are packed by class c = f mod 4 so every combine has uniform
taps across its 128 partitions:
    tiles 0-3: re classes c0(f<512),c1,c2,c3 ; tile 4: [re f=512; im c0] ;
    tiles 5-7: im classes c1,c2,c3.
Every K-chunk k then combines as U_k[s] = G_k[s] +/- G_k[s-2] where G_k is a
single two-operand shift-add:
    k=0: G=X0[s]+X0[s-1] (+)   k=2: G=X2[s]-X2[s-1] (+)
    k=4: G=X4[s]+X4[s-1] (+)   k=6: G=X6[s]-X6[s-1] (+)
    k=1: G=X1[s]-X5[s-1] (-)   k=5: G=X5[s]+X1[s-1] (-)
    k=3: G=X3[s]+X7[s-1] (-)   k=7: G=X7[s]-X3[s-1] (-)
2 elementwise ops per chunk, all on the Vector engine (GpSimd shares its
SBUF ports and is net-negative when run concurrently); chunk 0's second op
is folded into the PE as an extra accumulation pass to balance the two
engines. The PE consumes U chunks in arrival order with full-batch K-sweeps
over 8 PSUM banks of [128, 512] (2 s-tiles/bank, one bank-reset per batch).

Window-sum normalization: 0.25 folded into the basis; per-partition fixup
scales on the two edge s-tiles (3/2/1 valid frames).

All data bf16 (PSUM accumulates fp32); validated rel-err vs reference 3.7e-3.
Sharding: pure data parallel, 2 batches per core.
"""

import numpy as np
import ml_dtypes

import concourse.bass as bass
import concourse.mybir as mybir
from concourse.tile import TileContext
from concourse import bacc, bass_utils

N_FFT = 1024
HOP = 256
B = 16
NFREQ = 513
T = 2000
NCORES = 8
NB = B // NCORES          # batches per core
T2 = 2052                 # X cols: t = -1..2050 at col t+1; valid t=0..1999
S = 2048                  # U cols: s = 2..2049 at col s-2; out keeps s=2..2002
OUT_SEGS = 2001
OUT_LEN = OUT_SEGS * HOP  # 512256
# Chunk 0 (folded) first so the PE starts immediately; the two-tile c1/c3
# pairs arrive mid-stream; a single-tile chunk arrives LAST so the serial
# chain behind the final X tile is just G+U+one sweep (not the 4-op c3 pair).
DMAORDER = [0, 1, 5, 3, 7, 4, 2, 6]   # X-chunk DMA issue order
KORDER = [0, 1, 5, 3, 7, 4, 2, 6]     # matmul K-sweep order (= U arrival order)

F32 = mybir.dt.float32
BF16 = mybir.dt.bfloat16
NP_BF16 = ml_dtypes.bfloat16

_f = np.arange(NFREQ)
_IDX = {c: _f[_f % 4 == c] for c in range(4)}
_C0_RE = _IDX[0][:128]     # f = 0,4,...,508
_C0_IM = _IDX[0][1:-1]     # f = 4,...,508


def _prep_x(stft: np.ndarray) -> np.ndarray:
    """(16,513,2000,2) f32 -> (16, 8, 128, T2) bf16 class-packed, t zero-padded."""
    re, im = stft[..., 0], stft[..., 1]
    X = np.zeros((B, 8, 128, T2), np.float32)
    X[:, 0, :, 1:T + 1] = re[:, _C0_RE]
    X[:, 1, :, 1:T + 1] = re[:, _IDX[1]]
    X[:, 2, :, 1:T + 1] = re[:, _IDX[2]]
    X[:, 3, :, 1:T + 1] = re[:, _IDX[3]]
    X[:, 4, 0, 1:T + 1] = re[:, 512]
    X[:, 4, 1:128, 1:T + 1] = im[:, _C0_IM]
    X[:, 5, :, 1:T + 1] = im[:, _IDX[1]]
    X[:, 6, :, 1:T + 1] = im[:, _IDX[2]]
    X[:, 7, :, 1:T + 1] = im[:, _IDX[3]]
    return np.ascontiguousarray(X.astype(NP_BF16))


def _make_basis2() -> np.ndarray:
    """(8, 128, 256) basis tiles matching the class packing; 0.25 wss folded."""
    a32 = np.float32(2.0 * np.pi / N_FFT)
    fv = np.arange(NFREQ, dtype=np.float32)
    rv = np.arange(HOP, dtype=np.float32)
    t1 = (a32 * fv).astype(np.float32)
    ang = (t1[:, None] * rv[None, :]).astype(np.float32)
    w = np.full(NFREQ, 2.0, np.float32)
    w[0] = 1.0
    w[512] = 1.0
    Cb = (np.cos(ang) / np.float32(N_FFT)) * w[:, None] * np.float32(0.25)
    Sb = (-np.sin(ang) / np.float32(N_FFT)) * w[:, None] * np.float32(0.25)
    Bt = np.zeros((10, 128, HOP), np.float32)
    Bt[0] = Cb[_C0_RE]
    Bt[1] = Cb[_IDX[1]]
    Bt[2] = Cb[_IDX[2]]
    Bt[3] = Cb[_IDX[3]]
    Bt[4, 0] = Cb[512]
    Bt[4, 1:128] = Sb[_C0_IM]
    Bt[5] = Sb[_IDX[1]]
    Bt[6] = Sb[_IDX[2]]
    Bt[7] = Sb[_IDX[3]]
    # slots 8, 9: negated basis for the PE-folded chunks 3 and 7, whose
    # U = G[s] - G[s-2] second pass accumulates with flipped sign.
    Bt[8] = -Bt[3]
    Bt[9] = -Bt[7]
    return np.ascontiguousarray(Bt.astype(NP_BF16))


def _make_scales() -> np.ndarray:
    """(128, 2) per-partition wss fixup (on top of the 0.25 folded into basis).

    col 0 -> first s-tile (s = 2..129): s=2 has 3 frames -> 4/3.
    col 1 -> last s-tile (s = 1922..2049): s=2000 -> 4/3, 2001 -> 2, 2002 -> 4.
    """
    sc = np.ones((128, 2), np.float32)
    sc[0, 0] = np.float32(4.0) / np.float32(3.0)
    sc[78, 1] = np.float32(4.0) / np.float32(3.0)
    sc[79, 1] = 2.0
    sc[80, 1] = 4.0
    return sc


def _build_nc() -> bass.Bass:
    nc = bacc.Bacc()
    x_in = nc.dram_tensor("x_in", [NB, 8, 128, T2], BF16, kind="ExternalInput")
    basis_in = nc.dram_tensor("basis_in", [10, 128, HOP], BF16, kind="ExternalInput")
    scale_in = nc.dram_tensor("scale_in", [128, 2], F32, kind="ExternalInput")
    # out[b, p, st*256 + r]: segment (st*128 + p), sample r. One DMA per batch
    # with 8KB contiguous per partition row.
    out = nc.dram_tensor("out", [NB, 128, 16 * HOP], BF16, kind="ExternalOutput")

    with TileContext(nc) as tc:
        with (
            tc.tile_pool(name="xp", bufs=1) as x_pool,
            tc.tile_pool(name="up", bufs=1) as u_pool,
            tc.tile_pool(name="scr", bufs=1) as scr_pool,
            tc.tile_pool(name="bp", bufs=1) as b_pool,
            tc.tile_pool(name="sp", bufs=1) as s_pool,
            tc.tile_pool(name="ev", bufs=1) as ev_pool,
            tc.tile_pool(name="ps", bufs=1, space="PSUM") as psum_pool,
        ):
            # X chunks on the Sync HWDGE queue (a single queue feeds the DMA
            # engines at full rate; splitting across queues measured slower);
            # basis + scale on the ACT HWDGE queue in parallel.
            x_sb = [[None] * 8 for _ in range(NB)]
            for b in range(NB):
                for k in DMAORDER:
                    xt = x_pool.tile([128, T2], BF16, name=f"x{b}_{k}", tag=f"x{b}_{k}")
                    if k == 0:
                        nc.sync.dma_start(xt[:, :], x_in[b, k])
                    else:
                        # cols beyond 2019 are never read for non-folded
                        # chunks -- skip them to shorten the input stream
                        nc.sync.dma_start(xt[:, 0:2020], x_in[b, k][:, 0:2020])
                    x_sb[b][k] = xt

            scale_sb = s_pool.tile([128, 2], F32, name="scale_sb", tag="scale_sb")
            scale_wu = s_pool.tile([128, 2], F32, name="scale_wu", tag="scale_wu")
            nc.scalar.dma_start(scale_sb[:, :], scale_in[:, :])
            basis_sb = [None] * 10
            for k in list(KORDER) + [8, 9]:
                bt = b_pool.tile([128, HOP], BF16, name=f"bas{k}", tag=f"bas{k}")
                nc.scalar.dma_start(bt[:, :], basis_in[k])
                basis_sb[k] = bt
            # ACT warm-up read so later activations skip the table load.
            nc.scalar.copy(scale_wu[:, :], scale_sb[:, :])

            # Frame combine, 2 ops per K-chunk: G then U = G[s] +/- G[s-2].
            # G[:, cq] pairs X[t=cq] with X[t=cq-1]; U[:, cs] is segment cs+2.
            # All on DVE: GpSimd 2-input elementwise contends with DVE for
            # SBUF ports (measured ~4x mutual slowdown), so it stays idle.
            # PE-folded chunks: DVE produces only G; the matmul runs two
            # passes (G[s+2-slice] with basis, G[s-slice] with -basis).
            # Folding trades -4.9us DVE for +6.8us PE, so only the earliest
            # chunk folds -- its extra PE passes fill the PE's initial idle
            # while it waits for the U stream, without growing the tail.
            FOLDED = (0,)
            u_sb = [[None] * 8 for _ in range(NB)]
            g_sb = [[None] * 8 for _ in range(NB)]
            for b in range(NB):
                for k in range(8):
                    if k not in FOLDED:
                        ut = u_pool.tile(
                            [128, S], BF16, name=f"u{b}_{k}", tag=f"u{b}_{k}"
                        )
                        # zero the never-combined tail (read by the st15
                        # stationary, discarded rows) on the idle GpSimd
                        # tail starts on a 32B SBUF line boundary so the DVE
                        # combine write and this memset never share a line
                        nc.gpsimd.memset(ut[:, 2016:S], 0.0)
                        u_sb[b][k] = ut

            def emit_g(b, k, ka, kb, g_add):
                # Folded chunks get per-batch scratch so batch 1's combine
                # never WAR-stalls on batch 0's late PE reads. Non-folded
                # chunks only need G cols [0:2004] (U stops at seg 2003;
                # later psum rows are discarded by the host).
                tag = f"g{b}_{k}" if k in FOLDED else f"g{k}"
                g = scr_pool.tile([128, T2], BF16, name=tag, tag=tag)
                op = nc.vector.tensor_add if g_add else nc.vector.tensor_sub
                w = 2050 if k in FOLDED else 2018
                op(g[:, 0:w], x_sb[b][ka][:, 1 : w + 1], x_sb[b][kb][:, 0:w])
                g_sb[b][k] = g
                return g

            def emit_u(b, k, g, u_add):
                op = nc.vector.tensor_add if u_add else nc.vector.tensor_sub
                op(u_sb[b][k][:, 0:2016], g[:, 2:2018], g[:, 0:2016])

            # (src_a, src_b, g_is_add, u_is_add); c3 pair (k=3,7) needs X7 so
            # it runs last, matching the X DMA order.
            SPEC = {
                0: (0, 0, True, True),
                6: (6, 6, False, True),
                4: (4, 4, True, True),
                2: (2, 2, False, True),
                1: (1, 5, False, False),
                5: (5, 1, True, False),
                3: (3, 7, True, False),
                7: (7, 3, False, False),
            }
            for b in range(NB):
                for k in KORDER:
                    sa, sb_, ga, ua = SPEC[k]
                    g = emit_g(b, k, sa, sb_, ga)
                    if k not in FOLDED:
                        emit_u(b, k, g, ua)

            # Matmul: full-batch K-sweeps; PSUM bank i holds s-tiles (2i, 2i+1).
            # A matmul 'start' zeroes the WHOLE bank, so only the very first
            # matmul touching a bank carries start=True; the upper half's first
            # accumulation lands on the bank-wide zero.
            for b in range(NB):
                pss = [
                    psum_pool.tile([128, 2 * HOP], F32, name=f"ps{i}", tag=f"ps{i}")
                    for i in range(8)
                ]
                for ki, k in enumerate(KORDER):
                    for i in range(8):
                        for hf in range(2):
                            st = 2 * i + hf
                            out_ap = pss[i][:, HOP * hf : HOP * (hf + 1)]
                            if k in FOLDED:
                                # second-pass rhs: plain basis for add-chunks
                                # (0,6,4,2), negated (slots 8,9) for 3,7
                                neg = basis_sb[{3: 8, 7: 9}.get(k, k)]
                                g = g_sb[b][k]
                                nc.tensor.matmul(
                                    out_ap,
                                    g[:, 128 * st + 2 : 128 * st + 130],
                                    basis_sb[k][:, :],
                                    start=(ki == 0 and hf == 0),
                                    stop=False,
                                    skip_group_check=True,
                                )
                                nc.tensor.matmul(
                                    out_ap,
                                    g[:, 128 * st : 128 * st + 128],
                                    neg[:, :],
                                    start=False,
                                    stop=(ki == 7),
                                    skip_group_check=True,
                                )
                            else:
                                nc.tensor.matmul(
                                    out_ap,
                                    u_sb[b][k][:, 128 * st : 128 * st + 128],
                                    basis_sb[k][:, :],
                                    start=(ki == 0 and hf == 0),
                                    stop=(ki == 7),
                                    skip_group_check=True,
                                )
                ev = ev_pool.tile([128, 16 * HOP], BF16, name=f"ev{b}", tag=f"ev{b}")
                # Evict: batch 0 all on ACT (DVE is still combining); the last
                # batch drains banks 7->0, splitting onto the now-idle DVE,
                # and issues its first-ready quarter from the idle Sync engine
                # so the tail is not serialized on one engine.
                last = b == NB - 1

                def evict(i, eng_v):
                    lo, hi = 512 * i, 512 * i + 256
                    if i == 0:
                        if eng_v:
                            nc.vector.tensor_scalar_mul(
                                ev[:, lo:hi], pss[i][:, 0:HOP], scale_sb[:, 0:1]
                            )
                            nc.vector.tensor_copy(ev[:, hi : hi + 256], pss[i][:, HOP:])
                        else:
                            nc.scalar.mul(
                                ev[:, lo:hi], pss[i][:, 0:HOP], scale_sb[:, 0:1]
                            )
                            nc.scalar.copy(ev[:, hi : hi + 256], pss[i][:, HOP:])
                    elif i == 7:
                        if eng_v:
                            nc.vector.tensor_copy(ev[:, lo:hi], pss[i][:, 0:HOP])
                            nc.vector.tensor_scalar_mul(
                                ev[:, hi : hi + 256], pss[i][:, HOP:], scale_sb[:, 1:2]
                            )
                        else:
                            nc.scalar.copy(ev[:, lo:hi], pss[i][:, 0:HOP])
                            nc.scalar.mul(
                                ev[:, hi : hi + 256], pss[i][:, HOP:], scale_sb[:, 1:2]
                            )
                    elif eng_v:
                        nc.vector.tensor_copy(ev[:, lo : lo + 512], pss[i][:, :])
                    else:
                        nc.scalar.copy(ev[:, lo : lo + 512], pss[i][:, :])

                def quarter(qi, eng):
                    q0 = 1024 * qi
                    eng.dma_start(out[b][:, q0 : q0 + 1024], ev[:, q0 : q0 + 1024])

                if not last:
                    # batch 0's stores wait (via an ACT-serial dummy read)
                    # for the last X tile so their transfers never steal
                    # fabric bandwidth from the input stream
                    for i in range(8):
                        evict(i, eng_v=False)
                    gate = s_pool.tile([128, 1], BF16, name="gate", tag="gate")
                    nc.scalar.copy(gate[:, :], x_sb[NB - 1][DMAORDER[-1]][:, 0:1])
                    for qi in range(4):
                        quarter(qi, nc.scalar)
                else:
                    for i in range(8):
                        evict(i, eng_v=(i >= 4))
                        if i % 2 == 1:
                            quarter(i // 2, nc.scalar)
    nc.finalize()
    return nc


def _run(inputs: dict, trace: bool = False):
    stft = np.asarray(inputs["stft_matrix"], dtype=np.float32)
    X = _prep_x(stft)
    basis = _make_basis2()
    scales = _make_scales()
    in_maps = [
        {"x_in": X[NB * c : NB * (c + 1)], "basis_in": basis, "scale_in": scales}
        for c in range(NCORES)
    ]
    nc = _build_nc()
    res = bass_utils.run_bass_kernel_spmd(
        nc, in_maps, core_ids=list(range(NCORES)), trace=trace
    )
    outs = []
    for c in range(NCORES):
        o = res.results[c]["out"]  # (NB, 128, 4096) bf16
        o = np.asarray(o, dtype=np.float32).reshape(NB, 128, 16, HOP)
        o = o.transpose(0, 2, 1, 3)  # (NB, st, p, r)
        outs.append(o.reshape(NB, 2048 * HOP)[:, :OUT_LEN])
    return np.concatenate(outs, axis=0), res


def kernel(**inputs) -> np.ndarray:
    out, _ = _run(inputs, trace=False)
    return out


# revision 40
# speedup vs baseline: 1.0152x; 1.0152x over previous
"""Inverse STFT (nn_InverseSTFT) as a Bass/Tile kernel on 8 TRN2 NeuronCores.

Math
----
Reference: full spectrum via conjugate symmetry (F = 1024), IDFT to ytmp,
overlap_add(hop=256), window-sum normalize, trim n_fft//2.

This kernel folds BOTH the conjugate symmetry AND the overlap-add into the
basis. With w = 256j + r the DFT phase satisfies
    e^{i 2pi f (256j+r)/1024} = i^{f j} * e^{i 2pi f r/1024},
so the 4-frame overlap-add sum becomes a per-frequency 4-tap filter along
frames with coefficients i^{f j} in {1, i, -1, -i} (fixed per class
c = f mod 4), followed by ONE K=1024 matmul per output segment:
    y[256 s + r] = sum_f cosb[f,r]*U[f,s] + sinb[f,r]*V[f,s]
i.e. 1024 MACs per output sample instead of 4096 (4x less PE work).

Frequencies are packed by class c = f mod 4 so every combine has uniform
taps across its 128 partitions:
    tiles 0-3: re classes c0(f<512),c1,c2,c3 ; tile 4: [re f=512; im c0] ;
    tiles 5-7: im classes c1,c2,c3.
Every K-chunk k then combines as U_k[s] = G_k[s] +/- G_k[s-2] where G_k is a
single two-operand shift-add:
    k=0: G=X0[s]+X0[s-1] (+)   k=2: G=X2[s]-X2[s-1] (+)
    k=4: G=X4[s]+X4[s-1] (+)   k=6: G=X6[s]-X6[s-1] (+)
    k=1: G=X1[s]-X5[s-1] (-)   k=5: G=X5[s]+X1[s-1] (-)
    k=3: G=X3[s]+X7[s-1] (-)   k=7: G=X7[s]-X3[s-1] (-)
2 elementwise ops per chunk, all on the Vector engine (GpSimd shares its
SBUF ports and is net-negative when run concurrently); chunk 0's second op
is folded into the PE as an extra accumulation pass to balance the two
engines. The PE consumes U chunks in arrival order with full-batch K-sweeps
over 8 PSUM banks of [128, 512] (2 s-tiles/bank, one bank-reset per batch).

Window-sum normalization: 0.25 folded into the basis; per-partition fixup
scales on the two edge s-tiles (3/2/1 valid frames).

All data bf16 (PSUM accumulates fp32); validated rel-err vs reference 3.7e-3.
Sharding: pure data parallel, 2 batches per core.
"""

import numpy as np
import ml_dtypes

import concourse.bass as bass
import concourse.mybir as mybir
from concourse.tile import TileContext
from concourse import bacc, bass_utils

N_FFT = 1024
HOP = 256
B = 16
NFREQ = 513
T = 2000
NCORES = 8
NB = B // NCORES          # batches per core
T2 = 2052                 # X cols: t = -1..2050 at col t+1; valid t=0..1999
S = 2048                  # U cols: s = 2..2049 at col s-2; out keeps s=2..2002
OUT_SEGS = 2001
OUT_LEN = OUT_SEGS * HOP  # 512256
# Single-tile chunk 6 arrives LAST: the c3 pair's 4-op chain then overlaps
# the final tile's transfer, and the post-last-tile chain is just G6+U6+sweep.
DMAORDER = [0, 4, 2, 1, 5, 3, 7, 6]   # X-chunk DMA issue order
KORDER = [0, 4, 2, 1, 5, 3, 7, 6]     # matmul K-sweep order (= U arrival order)

F32 = mybir.dt.float32
BF16 = mybir.dt.bfloat16
NP_BF16 = ml_dtypes.bfloat16

_f = np.arange(NFREQ)
_IDX = {c: _f[_f % 4 == c] for c in range(4)}
_C0_RE = _IDX[0][:128]     # f = 0,4,...,508
_C0_IM = _IDX[0][1:-1]     # f = 4,...,508


def _prep_x(stft: np.ndarray) -> np.ndarray:
    """(16,513,2000,2) f32 -> (16, 8, 128, T2) bf16 class-packed, t zero-padded."""
    re, im = stft[..., 0], stft[..., 1]
    X = np.zeros((B, 8, 128, T2), np.float32)
    X[:, 0, :, 1:T + 1] = re[:, _C0_RE]
    X[:, 1, :, 1:T + 1] = re[:, _IDX[1]]
    X[:, 2, :, 1:T + 1] = re[:, _IDX[2]]
    X[:, 3, :, 1:T + 1] = re[:, _IDX[3]]
    X[:, 4, 0, 1:T + 1] = re[:, 512]
    X[:, 4, 1:128, 1:T + 1] = im[:, _C0_IM]
    X[:, 5, :, 1:T + 1] = im[:, _IDX[1]]
    X[:, 6, :, 1:T + 1] = im[:, _IDX[2]]
    X[:, 7, :, 1:T + 1] = im[:, _IDX[3]]
    return np.ascontiguousarray(X.astype(NP_BF16))


def _make_basis2() -> np.ndarray:
    """(8, 128, 256) basis tiles matching the class packing; 0.25 wss folded."""
    a32 = np.float32(2.0 * np.pi / N_FFT)
    fv = np.arange(NFREQ, dtype=np.float32)
    rv = np.arange(HOP, dtype=np.float32)
    t1 = (a32 * fv).astype(np.float32)
    ang = (t1[:, None] * rv[None, :]).astype(np.float32)
    w = np.full(NFREQ, 2.0, np.float32)
    w[0] = 1.0
    w[512] = 1.0
    Cb = (np.cos(ang) / np.float32(N_FFT)) * w[:, None] * np.float32(0.25)
    Sb = (-np.sin(ang) / np.float32(N_FFT)) * w[:, None] * np.float32(0.25)
    Bt = np.zeros((10, 128, HOP), np.float32)
    Bt[0] = Cb[_C0_RE]
    Bt[1] = Cb[_IDX[1]]
    Bt[2] = Cb[_IDX[2]]
    Bt[3] = Cb[_IDX[3]]
    Bt[4, 0] = Cb[512]
    Bt[4, 1:128] = Sb[_C0_IM]
    Bt[5] = Sb[_IDX[1]]
    Bt[6] = Sb[_IDX[2]]
    Bt[7] = Sb[_IDX[3]]
    # slots 8, 9: negated basis for the PE-folded chunks 3 and 7, whose
    # U = G[s] - G[s-2] second pass accumulates with flipped sign.
    Bt[8] = -Bt[3]
    Bt[9] = -Bt[7]
    return np.ascontiguousarray(Bt.astype(NP_BF16))


def _make_scales() -> np.ndarray:
    """(128, 2) per-partition wss fixup (on top of the 0.25 folded into basis).

    col 0 -> first s-tile (s = 2..129): s=2 has 3 frames -> 4/3.
    col 1 -> last s-tile (s = 1922..2049): s=2000 -> 4/3, 2001 -> 2, 2002 -> 4.
    """
    sc = np.ones((128, 2), np.float32)
    sc[0, 0] = np.float32(4.0) / np.float32(3.0)
    sc[78, 1] = np.float32(4.0) / np.float32(3.0)
    sc[79, 1] = 2.0
    sc[80, 1] = 4.0
    return sc


def _build_nc() -> bass.Bass:
    nc = bacc.Bacc()
    x_in = nc.dram_tensor("x_in", [NB, 8, 128, T2], BF16, kind="ExternalInput")
    basis_in = nc.dram_tensor("basis_in", [10, 128, HOP], BF16, kind="ExternalInput")
    scale_in = nc.dram_tensor("scale_in", [128, 2], F32, kind="ExternalInput")
    # out[b, p, st*256 + r]: segment (st*128 + p), sample r. One DMA per batch
    # with 8KB contiguous per partition row.
    out = nc.dram_tensor("out", [NB, 128, 16 * HOP], BF16, kind="ExternalOutput")

    with TileContext(nc) as tc:
        with (
            tc.tile_pool(name="xp", bufs=1) as x_pool,
            tc.tile_pool(name="up", bufs=1) as u_pool,
            tc.tile_pool(name="scr", bufs=1) as scr_pool,
            tc.tile_pool(name="bp", bufs=1) as b_pool,
            tc.tile_pool(name="sp", bufs=1) as s_pool,
            tc.tile_pool(name="ev", bufs=1) as ev_pool,
            tc.tile_pool(name="ps", bufs=1, space="PSUM") as psum_pool,
        ):
            # X chunks on the Sync HWDGE queue (a single queue feeds the DMA
            # engines at full rate; splitting across queues measured slower);
            # basis + scale on the ACT HWDGE queue in parallel.
            x_sb = [[None] * 8 for _ in range(NB)]
            for b in range(NB):
                for k in DMAORDER:
                    xt = x_pool.tile([128, T2], BF16, name=f"x{b}_{k}", tag=f"x{b}_{k}")
                    if k == 0:
                        nc.sync.dma_start(xt[:, :], x_in[b, k])
                    else:
                        # cols beyond 2019 are never read for non-folded
                        # chunks -- skip them to shorten the input stream
                        nc.sync.dma_start(xt[:, 0:2020], x_in[b, k][:, 0:2020])
                    x_sb[b][k] = xt

            scale_sb = s_pool.tile([128, 2], F32, name="scale_sb", tag="scale_sb")
            scale_wu = s_pool.tile([128, 2], F32, name="scale_wu", tag="scale_wu")
            nc.scalar.dma_start(scale_sb[:, :], scale_in[:, :])
            basis_sb = [None] * 10
            for k in list(KORDER) + [8, 9]:
                bt = b_pool.tile([128, HOP], BF16, name=f"bas{k}", tag=f"bas{k}")
                nc.scalar.dma_start(bt[:, :], basis_in[k])
                basis_sb[k] = bt
            # ACT warm-up read so later activations skip the table load.
            nc.scalar.copy(scale_wu[:, :], scale_sb[:, :])

            # Frame combine, 2 ops per K-chunk: G then U = G[s] +/- G[s-2].
            # G[:, cq] pairs X[t=cq] with X[t=cq-1]; U[:, cs] is segment cs+2.
            # All on DVE: GpSimd 2-input elementwise contends with DVE for
            # SBUF ports (measured ~4x mutual slowdown), so it stays idle.
            # PE-folded chunks: DVE produces only G; the matmul runs two
            # passes (G[s+2-slice] with basis, G[s-slice] with -basis).
            # Folding trades -4.9us DVE for +6.8us PE, so only the earliest
            # chunk folds -- its extra PE passes fill the PE's initial idle
            # while it waits for the U stream, without growing the tail.
            FOLDED = (0,)
            u_sb = [[None] * 8 for _ in range(NB)]
            g_sb = [[None] * 8 for _ in range(NB)]
            for b in range(NB):
                for k in range(8):
                    if k not in FOLDED:
                        ut = u_pool.tile(
                            [128, S], BF16, name=f"u{b}_{k}", tag=f"u{b}_{k}"
                        )
                        # zero the never-combined tail (read by the st15
                        # stationary, discarded rows) on the idle GpSimd
                        # tail starts on a 32B SBUF line boundary so the DVE
                        # combine write and this memset never share a line
                        nc.gpsimd.memset(ut[:, 2016:S], 0.0)
                        u_sb[b][k] = ut

            def emit_g(b, k, ka, kb, g_add):
                # Folded chunks get per-batch scratch so batch 1's combine
                # never WAR-stalls on batch 0's late PE reads. Non-folded
                # chunks only need G cols [0:2004] (U stops at seg 2003;
                # later psum rows are discarded by the host).
                tag = f"g{b}_{k}" if k in FOLDED else f"g{k}"
                g = scr_pool.tile([128, T2], BF16, name=tag, tag=tag)
                op = nc.vector.tensor_add if g_add else nc.vector.tensor_sub
                w = 2050 if k in FOLDED else 2018
                op(g[:, 0:w], x_sb[b][ka][:, 1 : w + 1], x_sb[b][kb][:, 0:w])
                g_sb[b][k] = g
                return g

            def emit_u(b, k, g, u_add):
                op = nc.vector.tensor_add if u_add else nc.vector.tensor_sub
                op(u_sb[b][k][:, 0:2016], g[:, 2:2018], g[:, 0:2016])

            # (src_a, src_b, g_is_add, u_is_add); c3 pair (k=3,7) needs X7 so
            # it runs last, matching the X DMA order.
            SPEC = {
                0: (0, 0, True, True),
                6: (6, 6, False, True),
                4: (4, 4, True, True),
                2: (2, 2, False, True),
                1: (1, 5, False, False),
                5: (5, 1, True, False),
                3: (3, 7, True, False),
                7: (7, 3, False, False),
            }
            for b in range(NB):
                for k in KORDER:
                    sa, sb_, ga, ua = SPEC[k]
                    g = emit_g(b, k, sa, sb_, ga)
                    if k not in FOLDED:
                        emit_u(b, k, g, ua)

            # Matmul: full-batch K-sweeps; PSUM bank i holds s-tiles (2i, 2i+1).
            # A matmul 'start' zeroes the WHOLE bank, so only the very first
            # matmul touching a bank carries start=True; the upper half's first
            # accumulation lands on the bank-wide zero.
            for b in range(NB):
                pss = [
                    psum_pool.tile([128, 2 * HOP], F32, name=f"ps{i}", tag=f"ps{i}")
                    for i in range(8)
                ]
                for ki, k in enumerate(KORDER):
                    for i in range(8):
                        for hf in range(2):
                            st = 2 * i + hf
                            out_ap = pss[i][:, HOP * hf : HOP * (hf + 1)]
                            if k in FOLDED:
                                # second-pass rhs: plain basis for add-chunks
                                # (0,6,4,2), negated (slots 8,9) for 3,7
                                neg = basis_sb[{3: 8, 7: 9}.get(k, k)]
                                g = g_sb[b][k]
                                nc.tensor.matmul(
                                    out_ap,
                                    g[:, 128 * st + 2 : 128 * st + 130],
                                    basis_sb[k][:, :],
                                    start=(ki == 0 and hf == 0),
                                    stop=False,
                                    skip_group_check=True,
                                )
                                nc.tensor.matmul(
                                    out_ap,
                                    g[:, 128 * st : 128 * st + 128],
                                    neg[:, :],
                                    start=False,
                                    stop=(ki == 7),
                                    skip_group_check=True,
                                )
                            else:
                                nc.tensor.matmul(
                                    out_ap,
                                    u_sb[b][k][:, 128 * st : 128 * st + 128],
                                    basis_sb[k][:, :],
                                    start=(ki == 0 and hf == 0),
                                    stop=(ki == 7),
                                    skip_group_check=True,
                                )
                ev = ev_pool.tile([128, 16 * HOP], BF16, name=f"ev{b}", tag=f"ev{b}")
                # Evict: batch 0 all on ACT (DVE is still combining); the last
                # batch drains banks 7->0, splitting onto the now-idle DVE,
                # and issues its first-ready quarter from the idle Sync engine
                # so the tail is not serialized on one engine.
                last = b == NB - 1

                def evict(i, eng_v):
                    lo, hi = 512 * i, 512 * i + 256
                    if i == 0:
                        if eng_v:
                            nc.vector.tensor_scalar_mul(
                                ev[:, lo:hi], pss[i][:, 0:HOP], scale_sb[:, 0:1]
                            )
                            nc.vector.tensor_copy(ev[:, hi : hi + 256], pss[i][:, HOP:])
                        else:
                            nc.scalar.mul(
                                ev[:, lo:hi], pss[i][:, 0:HOP], scale_sb[:, 0:1]
                            )
                            nc.scalar.copy(ev[:, hi : hi + 256], pss[i][:, HOP:])
                    elif i == 7:
                        if eng_v:
                            nc.vector.tensor_copy(ev[:, lo:hi], pss[i][:, 0:HOP])
                            nc.vector.tensor_scalar_mul(
                                ev[:, hi : hi + 256], pss[i][:, HOP:], scale_sb[:, 1:2]
                            )
                        else:
                            nc.scalar.copy(ev[:, lo:hi], pss[i][:, 0:HOP])
                            nc.scalar.mul(
                                ev[:, hi : hi + 256], pss[i][:, HOP:], scale_sb[:, 1:2]
                            )
                    elif eng_v:
                        nc.vector.tensor_copy(ev[:, lo : lo + 512], pss[i][:, :])
                    else:
                        nc.scalar.copy(ev[:, lo : lo + 512], pss[i][:, :])

                def quarter(qi, eng):
                    q0 = 1024 * qi
                    eng.dma_start(out[b][:, q0 : q0 + 1024], ev[:, q0 : q0 + 1024])

                if not last:
                    # batch 0's stores wait (via an ACT-serial dummy read)
                    # for the last X tile so their transfers never steal
                    # fabric bandwidth from the input stream
                    for i in range(8):
                        evict(i, eng_v=False)
                    gate = s_pool.tile([128, 1], BF16, name="gate", tag="gate")
                    nc.scalar.copy(gate[:, :], x_sb[NB - 1][DMAORDER[-1]][:, 0:1])
                    for qi in range(4):
                        quarter(qi, nc.scalar)
                else:
                    for i in range(8):
                        evict(i, eng_v=(i >= 4))
                        if i % 2 == 1:
                            quarter(i // 2, nc.scalar)
    nc.finalize()
    return nc


def _run(inputs: dict, trace: bool = False):
    stft = np.asarray(inputs["stft_matrix"], dtype=np.float32)
    X = _prep_x(stft)
    basis = _make_basis2()
    scales = _make_scales()
    in_maps = [
        {"x_in": X[NB * c : NB * (c + 1)], "basis_in": basis, "scale_in": scales}
        for c in range(NCORES)
    ]
    nc = _build_nc()
    res = bass_utils.run_bass_kernel_spmd(
        nc, in_maps, core_ids=list(range(NCORES)), trace=trace
    )
    outs = []
    for c in range(NCORES):
        o = res.results[c]["out"]  # (NB, 128, 4096) bf16
        o = np.asarray(o, dtype=np.float32).reshape(NB, 128, 16, HOP)
        o = o.transpose(0, 2, 1, 3)  # (NB, st, p, r)
        outs.append(o.reshape(NB, 2048 * HOP)[:, :OUT_LEN])
    return np.concatenate(outs, axis=0), res


def kernel(**inputs) -> np.ndarray:
    out, _ = _run(inputs, trace=False)
    return out
